# revision 30
# baseline (speedup 1.0000x reference)
"""TRN2 Bass kernel for nn_Attention_43396349559334.

Prefill attention layer: B=4 seqs x S=1024, H=2048, 16 q heads / 8 kv heads
(GQA rep 2), HD=128, weight-only-quantized projections (group 128), KV int8
quant-dequant roundtrip (group 8 along head dim), interleaved RoPE, causal.

Sharding over 8 cores: core c = 2*s + t -> sequence s (data parallel over the
4 sequences), TP half t (8 q heads + 4 kv heads per core; row-parallel wo with
host-side partial sum over TP pairs).

Numerics: q and v projections and q@k' scores run as single fp32r matmuls
(~17 effective mantissa bits at full fp16 PE rate for >=256-col streams).
The k projection keeps fp16 hi/lo pair matmuls (21-bit): k feeds the int8
cache quantization whose rounding decisions amplify small errors by a full
quantization step, so ~1e-5 relative accuracy is required there. P@v' and
wo are single fp16. All weight dequantization and the x hi/lo split happen
on the host; the xl half streams from DRAM per chunk and is never resident.
"""
import math
import numpy as np
from contextlib import ExitStack

import concourse.bass as bass
import concourse.bacc as bacc
import concourse.mybir as mybir
import concourse.tile as tile
from concourse.bass_utils import run_bass_kernel_spmd
from concourse.masks import make_identity, make_causal_mask

dt = mybir.dt
F32, F16, I32, F32R = dt.float32, dt.float16, dt.int32, dt.float32r
AF = mybir.ActivationFunctionType
OP = mybir.AluOpType

B, S, H = 4, 1024, 2048
NH, NKV, HD = 16, 8, 128
WG, CG = 128, 8
ROPE_THETA = 10000.0
TOK = S                  # tokens per core (one sequence)
NHC, NKVC = NH // 2, NKV // 2   # per-core heads: 8 q, 4 kv
KC = H // 128            # 16 contraction chunks
TC = TOK // 128          # 8 token chunks
PW = 512                 # projection piece width (outs per piece)
INVSQ = 1.0 / math.sqrt(HD)
NEG = -1e30


def build_kernel(nc):
    """Emit the per-core kernel."""
    # k path: x hi/lo and w_k hi/lo in f16 (host pre-split / pre-dequantized)
    xh16 = nc.declare_dram_parameter("xh16", [128, KC, TOK], F16, isOutput=False)
    xl16 = nc.declare_dram_parameter("xl16", [128, KC, TOK], F16, isOutput=False)
    wkh = nc.declare_dram_parameter("wkh", [128, KC, NKVC * HD], F16, isOutput=False)
    wkl = nc.declare_dram_parameter("wkl", [128, KC, NKVC * HD], F16, isOutput=False)
    # q/v path: f32 operands for fp32r matmuls (host pre-dequantized)
    x32 = nc.declare_dram_parameter("x32", [128, KC, TOK], F32R, isOutput=False)
    wq0 = nc.declare_dram_parameter("wq0", [128, KC, PW], F32R, isOutput=False)
    wq1 = nc.declare_dram_parameter("wq1", [128, KC, PW], F32R, isOutput=False)
    wv0 = nc.declare_dram_parameter("wv0", [128, KC, PW], F32R, isOutput=False)
    # wo: host pre-dequantized f16
    wo16d = nc.declare_dram_parameter("wo16d", [128, H // 128, NHC, 128], F16, isOutput=False)
    cosF = nc.declare_dram_parameter("cosF", [TOK, HD // 2], F32, isOutput=False)
    sinF = nc.declare_dram_parameter("sinF", [TOK, HD // 2], F32, isOutput=False)
    outT = nc.declare_dram_parameter("outT", [H, TOK], F32, isOutput=True)

    with tile.TileContext(nc) as tc, ExitStack() as top:
        const_p = top.enter_context(tc.tile_pool(name="const", bufs=1))
        small_p = top.enter_context(tc.tile_pool(name="small", bufs=2))
        stage_p = top.enter_context(tc.tile_pool(name="stage", bufs=2))
        store_p = top.enter_context(tc.tile_pool(name="store", bufs=1))

        # ---------------- constants ----------------
        ident16 = const_p.tile([128, 128], F16)
        make_identity(nc, ident16[:])
        ident32 = const_p.tile([128, 128], F32)
        make_identity(nc, ident32[:])
        cmask = const_p.tile([128, 128], F32)
        make_causal_mask(nc, cmask[:], mask_val=NEG)
        cosT = const_p.tile([128, TC, HD // 2], F32)   # [tok128, tchunk, 64]
        sinT = const_p.tile([128, TC, HD // 2], F32)
        nc.sync.dma_start(cosT[:], cosF[:].rearrange("(t p) d -> p t d", p=128))
        nc.sync.dma_start(sinT[:], sinF[:].rearrange("(t p) d -> p t d", p=128))

        # long-lived stores
        kT32 = store_p.tile([128, NKVC, TOK], F32R, tag="kT", bufs=1)  # [HD, kvh, tok]
        v16 = store_p.tile([128, TC, NKVC * HD], F16, tag="v16", bufs=1)
        x32s = store_p.tile([128, KC, TOK], F32R, tag="x32s", bufs=1)
        qT32 = store_p.tile([128, NHC, TOK], F32R, tag="qT", bufs=1)   # [HD, head, tok]

        # ---------- helpers ----------
        def rope(acc, t, width, out_tag):
            """acc: [128, width] f32 (PSUM ok) -> rot [128, width] f32 SBUF."""
            nh = width // HD
            rot = stage_p.tile([128, PW], F32, tag=out_tag, name="rot")
            v4 = lambda ap: ap.rearrange("p (h d two) -> p h d two", h=nh, two=2)
            te, to = v4(acc[:, :width])[:, :, :, 0], v4(acc[:, :width])[:, :, :, 1]
            re, ro = v4(rot[:, :width])[:, :, :, 0], v4(rot[:, :width])[:, :, :, 1]
            cos = cosT[:, t, :].unsqueeze(1).broadcast_to([128, nh, HD // 2])
            sin = sinT[:, t, :].unsqueeze(1).broadcast_to([128, nh, HD // 2])
            t1 = stage_p.tile([128, PW // 2], F32, tag="rope_t1", bufs=1, name="t1")
            t2 = stage_p.tile([128, PW // 2], F32, tag="rope_t2", bufs=1, name="t2")
            t1v = t1[:, :width // 2].rearrange("p (h d) -> p h d", h=nh)
            t2v = t2[:, :width // 2].rearrange("p (h d) -> p h d", h=nh)
            nc.vector.tensor_tensor(out=t1v, in0=to, in1=sin, op=OP.mult)
            nc.vector.tensor_tensor(out=t2v, in0=te, in1=cos, op=OP.mult)
            nc.vector.tensor_tensor(out=re, in0=t2v, in1=t1v, op=OP.subtract)
            nc.vector.tensor_tensor(out=t1v, in0=te, in1=sin, op=OP.mult)
            nc.vector.tensor_tensor(out=t2v, in0=to, in1=cos, op=OP.mult)
            nc.vector.tensor_tensor(out=ro, in0=t1v, in1=t2v, op=OP.add)
            return rot

        def quant(x32t, width, out_ap):
            """x32t: f32 tile [128, >=width]; out_ap: [128, ng, CG] view."""
            ng = width // CG
            xg = x32t[:, :width].rearrange("p (g c) -> p g c", c=CG)
            amax = small_p.tile([128, PW // CG], F32, tag="amax", name="amax")
            nc.vector.tensor_reduce(amax[:, :ng], xg, axis=mybir.AxisListType.X,
                                    op=OP.max, apply_absolute_value=True)
            s = small_p.tile([128, PW // CG], F32, tag="qs", name="s")
            nc.vector.tensor_scalar(out=s[:, :ng], in0=amax[:, :ng], scalar1=1.0 / 127.0,
                                    scalar2=1e-8, op0=OP.mult, op1=OP.add)
            rinv = small_p.tile([128, PW // CG], F32, tag="qrinv", name="rinv")
            nc.vector.reciprocal(rinv[:, :ng], s[:, :ng])
            y = stage_p.tile([128, PW], F32, tag="qy", name="y")
            nc.vector.tensor_tensor(out=y[:, :width].rearrange("p (g c) -> p g c", c=CG),
                                    in0=xg,
                                    in1=rinv[:, :ng].unsqueeze(2).broadcast_to([128, ng, CG]),
                                    op=OP.mult)
            lev = stage_p.tile([128, PW], I32, tag="qlev", bufs=1, name="lev")
            nc.scalar.copy(lev[:, :width], y[:, :width])
            levf = stage_p.tile([128, PW], F32, tag="qy", name="levf")
            nc.scalar.copy(levf[:, :width], lev[:, :width])
            nc.vector.tensor_tensor(out=out_ap,
                                    in0=levf[:, :width].rearrange("p (g c) -> p g c", c=CG),
                                    in1=s[:, :ng].unsqueeze(2).broadcast_to([128, ng, CG]),
                                    op=OP.mult)

        # ============ phases K and QV share psum pools ============
        with tc.tile_pool(name="ps_acc", bufs=1, space="PSUM") as ps_acc, \
             tc.tile_pool(name="ps_tr", bufs=1, space="PSUM") as ps_tr:
            # warm the PE (p-state ramp) while the first input DMAs land
            # (borrows the QV accumulator slots, long free by the time QV runs)
            junk = ps_acc.tile([128, PW], F32, tag="accq", bufs=2, name="junk")
            for _ in range(10):
                nc.tensor.matmul(junk[:, :128], ident32[:], ident32[:],
                                 start=True, stop=True)

            # ---- phase K: k projection, f16 hi/lo 3mm, g-outer over t-halves.
            # x hi/lo stream from DRAM in 4-chunk quarters in exact consumption
            # order (few, large DMA triggers: the sync engine issues triggers
            # serially, so trigger count and order gate the pipeline start).
            with tc.tile_pool(name="kphase", bufs=1) as kp:
                wkh_s = kp.tile([128, KC, NKVC * HD], F16)
                wkl_s = kp.tile([128, KC, NKVC * HD], F16)

                for half in range(2):
                    h0 = half * 512
                    ts = list(range(half * 4, half * 4 + 4))
                    accs = {}
                    for t in ts:
                        accs[t] = ps_acc.tile([128, PW], F32, tag="acck", bufs=4,
                                              name=f"acc{t}")
                    for gq in range(4):
                        gs = slice(gq * 4, gq * 4 + 4)
                        xhq = kp.tile([128, 4, 512], F16, tag="xhq", bufs=2, name="xhq")
                        xlq = kp.tile([128, 4, 512], F16, tag="xlq", bufs=2, name="xlq")
                        if half == 0:
                            nc.sync.dma_start(wkh_s[:, gs, :], wkh[:, gs, :])
                        nc.sync.dma_start(xhq[:], xh16[:, gs, h0:h0 + 512])
                        nc.sync.dma_start(xlq[:], xl16[:, gs, h0:h0 + 512])
                        if half == 0:
                            nc.sync.dma_start(wkl_s[:, gs, :], wkl[:, gs, :])
                        if half == 0:
                            # prefetch the fp32r x for phase QV behind phase-K input
                            nc.sync.dma_start(x32s[:, gs, :], x32[:, gs, :])
                        n = KC * 3
                        for gi in range(4):
                            g = gq * 4 + gi
                            for ti, t in enumerate(ts):
                                i = g * 3
                                lx_h = xhq[:, gi, ti * 128:(ti + 1) * 128]
                                lx_l = xlq[:, gi, ti * 128:(ti + 1) * 128]
                                nc.tensor.matmul(accs[t][:], lx_h, wkh_s[:, g, :],
                                                 start=(i == 0), stop=(i == n - 1))
                                nc.tensor.matmul(accs[t][:], lx_h, wkl_s[:, g, :],
                                                 start=False, stop=(i + 1 == n - 1))
                                nc.tensor.matmul(accs[t][:], lx_l, wkh_s[:, g, :],
                                                 start=False, stop=(i + 2 == n - 1))
                    for t in ts:
                        rot = rope(accs[t], t, PW, "rot")
                        ktmp = stage_p.tile([128, PW], F32, tag="ktmp", bufs=1, name="ktmp")
                        quant(rot, PW, ktmp[:].rearrange("p (g c) -> p g c", c=CG))
                        pt = ps_tr.tile([128, PW], F32, tag="pt", bufs=2, name="pt")
                        for j in range(NKVC):
                            nc.tensor.transpose(pt[:, j * 128:(j + 1) * 128],
                                                ktmp[:, j * 128:(j + 1) * 128], ident32[:])
                        nc.vector.tensor_copy(
                            kT32[:, :, t * 128:(t + 1) * 128],
                            pt[:].rearrange("p (j f) -> p j f", j=NKVC))

            # ---- phase QV: q/v projections (fp32r) ----
            with tc.tile_pool(name="wpiece", bufs=1) as w_p:
                pieces = [("q", 0, wq0), ("q", 1, wq1), ("v", 0, wv0)]
                for kind, p, w_dram in pieces:
                    wp = w_p.tile([128, KC, PW], F32R, tag="w32", bufs=2, name="wp")
                    nc.sync.dma_start(wp[:, :KC // 2, :], w_dram[:, :KC // 2, :])
                    nc.sync.dma_start(wp[:, KC // 2:, :], w_dram[:, KC // 2:, :])
                    for t in range(TC):
                        acc = ps_acc.tile([128, PW], F32, tag="accq", bufs=2, name="acc")
                        for g in range(KC):
                            nc.tensor.matmul(
                                acc[:],
                                x32s[:, g, t * 128:(t + 1) * 128],
                                wp[:, g, :],
                                start=(g == 0), stop=(g == KC - 1))
                        if kind == "q":
                            rot = rope(acc, t, PW, "rot")
                            pt = ps_tr.tile([128, PW], F32, tag="pt", bufs=2, name="pt")
                            for j in range(PW // 128):
                                nc.tensor.transpose(pt[:, j * 128:(j + 1) * 128],
                                                    rot[:, j * 128:(j + 1) * 128],
                                                    ident32[:])
                            nc.vector.tensor_copy(
                                qT32[:, 4 * p:4 * p + 4, t * 128:(t + 1) * 128],
                                pt[:].rearrange("p (j f) -> p j f", j=4))
                        else:
                            vstage = stage_p.tile([128, PW], F32, tag="rot", name="vstage")
                            nc.scalar.copy(vstage[:], acc[:])
                            quant(vstage, PW,
                                  v16[:, t, :].rearrange("p (g c) -> p g c", c=CG))

        # ============ phase A: attention with interleaved WO ============
        with tc.tile_pool(name="attnT", bufs=1) as at_p, \
             tc.tile_pool(name="wow", bufs=2) as wo_p:
            attnT = at_p.tile([128, NHC, TOK], F16)
            with tc.tile_pool(name="probs", bufs=3) as p_p, \
                 tc.tile_pool(name="ps_sc", bufs=2, space="PSUM") as psum_s, \
                 tc.tile_pool(name="ps_av", bufs=1, space="PSUM") as psum_v, \
                 tc.tile_pool(name="ps_pt", bufs=2, space="PSUM") as psum_pt, \
                 tc.tile_pool(name="ps_wo", bufs=1, space="PSUM") as psum_w:

                def attention_head(h, qi, pts_pair, h01, rsum2):
                    hkv = h // 2
                    L = (qi + 1) * 128
                    sc = psum_s.tile([128, TOK], F32, tag="scores", name="sc")
                    lq = qT32[:, h, qi * 128:(qi + 1) * 128]
                    for ci in range((L + 511) // 512):
                        c0, c1 = ci * 512, min(L, ci * 512 + 512)
                        nc.tensor.matmul(sc[:, c0:c1], lq, kT32[:, hkv, c0:c1],
                                         start=True, stop=True)
                    nc.vector.tensor_tensor(out=sc[:, L - 128:L], in0=sc[:, L - 128:L],
                                            in1=cmask[:], op=OP.add)
                    negm = small_p.tile([128, 1], F32, tag="negm", name="negm")
                    nc.vector.tensor_reduce(negm[:], sc[:, :L], axis=mybir.AxisListType.X,
                                            op=OP.max, negate=True)
                    bias = small_p.tile([128, 1], F32, tag="bias", name="bias")
                    nc.vector.tensor_scalar(out=bias[:], in0=negm[:], scalar1=INVSQ,
                                            scalar2=None, op0=OP.mult)
                    p16u = p_p.tile([128, TOK], F16, tag="p16u", bufs=4, name="p16u")
                    nc.scalar.activation(p16u[:, :L], sc[:, :L], AF.Exp,
                                         bias=bias[:], scale=INVSQ,
                                         accum_out=rsum2[:, h01:h01 + 1])
                    return p16u

                def normalize_transpose(p16u, qi, pts_pair, h01, rinv2):
                    L = (qi + 1) * 128
                    p16 = p_p.tile([128, TOK], F16, tag="p16", bufs=4, name="p16")
                    nc.vector.tensor_scalar(out=p16[:, :L], in0=p16u[:, :L],
                                            scalar1=rinv2[:, h01:h01 + 1],
                                            scalar2=None, op0=OP.mult)
                    ptp = psum_pt.tile([128, TC * 128], F16, tag="ptp", name="ptp")
                    for kc in range(qi + 1):
                        nc.tensor.transpose(ptp[:, kc * 128:(kc + 1) * 128],
                                            p16[:, kc * 128:(kc + 1) * 128], ident16[:])
                    nc.scalar.copy(
                        pts_pair[:, :qi + 1, h01, :],
                        ptp[:, :L].rearrange("p (k f) -> p k f", f=128))

                def wo_seg(hc, c0, w):
                    wo16 = wo_p.tile([128, NHC, 128], F16, tag="wo16", name="wo16")
                    nc.sync.dma_start(wo16[:], wo16d[:, hc, :, :])
                    po = psum_w.tile([128, 512], F32, tag="wo_out", name="po")
                    for g in range(NHC):
                        nc.tensor.matmul(po[:, :w], wo16[:, g, :],
                                         attnT[:, g, c0:c0 + w],
                                         start=(g == 0), stop=(g == NHC - 1))
                    pos = wo_p.tile([128, 512], F32, tag="wo_stage", bufs=3, name="pos")
                    nc.scalar.copy(pos[:, :w], po[:, :w])
                    nc.sync.dma_start(
                        outT[hc * 128:(hc + 1) * 128, c0:c0 + w], pos[:, :w])

                unit = 0
                for qi in (4, 5, 6, 7, 0, 1, 2, 3):
                    for hp in range(NHC // 2):
                        pts_pair = p_p.tile([128, TC, 2, 128], F16, tag="ptsp",
                                            bufs=4, name="pts_pair")
                        rsum2 = small_p.tile([128, 2], F32, tag="rsum", name="rsum2")
                        pa = attention_head(2 * hp, qi, pts_pair, 0, rsum2)
                        pb = attention_head(2 * hp + 1, qi, pts_pair, 1, rsum2)
                        rinv2 = small_p.tile([128, 2], F32, tag="rinv", name="rinv2")
                        nc.vector.reciprocal(rinv2[:], rsum2[:])
                        normalize_transpose(pa, qi, pts_pair, 0, rinv2)
                        normalize_transpose(pb, qi, pts_pair, 1, rinv2)
                        avp = psum_v.tile([128, 2, 128], F32, tag="avp", name="avp")
                        for kc in range(qi + 1):
                            nc.tensor.matmul(
                                avp[:].rearrange("p a b -> p (a b)"),
                                v16[:, kc, hp * HD:(hp + 1) * HD],
                                pts_pair[:, kc, :, :].rearrange("p a b -> p (a b)"),
                                start=(kc == 0), stop=(kc == qi))
                        nc.vector.tensor_copy(
                            attnT[:, 2 * hp:2 * hp + 2, qi * 128:(qi + 1) * 128],
                            avp[:])
                        # qi 4-7 ran first, so cols 512-1023 are final once the
                        # thin (qi 0-3) units start: spread WO half-1 through them
                        if qi < 4:
                            wo_seg(unit, 512, 512)
                            unit += 1
            # WO token-half-0 tail with the attention psum banks freed
            with tc.tile_pool(name="ps_wo2", bufs=3, space="PSUM") as psum_w:
                for hc in range(H // 128):
                    wo_seg(hc, 0, 512)


# ====================== host side ======================

_COMPILED = {}
TRACE = False
LAST_RESULTS = None


def _build():
    nc = bacc.Bacc("TRN2", target_bir_lowering=False, debug=False, num_devices=8)
    build_kernel(nc)
    nc.compile()
    return nc


def _prep_core_inputs(x, wqkv_q, wqkv_scale, wo_q, wo_scale, start_pos):
    """Build the 8 per-core input maps (numpy marshaling only)."""
    ins = []
    inv_freq = 1.0 / (ROPE_THETA ** (np.arange(0, HD, 2, dtype=np.float64) / HD))

    # full dequantized wqkv / wo in f32
    wqkv32 = wqkv_q.astype(np.float32) * np.repeat(wqkv_scale.astype(np.float32), WG, axis=1)
    wo32 = wo_q.astype(np.float32) * np.repeat(wo_scale.astype(np.float32), WG, axis=1)

    def arrange_T(a):
        """[out, H] f32 -> w^T arranged [128, KC, out] (partition = in-dim within chunk)."""
        aT = np.ascontiguousarray(a.T)                  # [H, out]
        return np.ascontiguousarray(
            aT.reshape(KC, 128, a.shape[0]).transpose(1, 0, 2))

    for c in range(8):
        s, t = c // 2, c % 2
        pos = (float(start_pos[s]) + np.arange(S, dtype=np.float64))[:, None] * inv_freq[None, :]
        cosF = np.cos(pos).astype(np.float32)
        sinF = np.sin(pos).astype(np.float32)

        xs = x[s * S:(s + 1) * S, :]                    # [1024, 2048]
        xT = np.ascontiguousarray(xs.T.astype(np.float32))   # [2048, 1024]

        qrows = slice(t * NHC * HD, (t + 1) * NHC * HD)
        krows = slice(NH * HD + t * NKVC * HD, NH * HD + (t + 1) * NKVC * HD)
        vrows = slice((NH + NKV) * HD + t * NKVC * HD, (NH + NKV) * HD + (t + 1) * NKVC * HD)

        wqT = arrange_T(wqkv32[qrows])                  # [128, KC, 1024]
        wkT = arrange_T(wqkv32[krows])                  # [128, KC, 512]
        wvT = arrange_T(wqkv32[vrows])                  # [128, KC, 512]
        wkTh = wkT.astype(np.float16)
        wkTl = (wkT - wkTh.astype(np.float32)).astype(np.float16)

        wo_cols = wo32[:, t * NHC * HD:(t + 1) * NHC * HD]      # [H, 1024]
        wo_T = wo_cols.T.astype(np.float16)                     # [1024, H]
        wo16d = np.ascontiguousarray(
            wo_T.reshape(NHC, 128, H // 128, 128).transpose(1, 2, 0, 3))  # [128, 16, 8, 128]

        xTh = xT.astype(np.float16)
        xTl = (xT - xTh.astype(np.float32)).astype(np.float16)

        def arrange_x(a):                               # [H, TOK] -> [128, KC, TOK]
            return np.ascontiguousarray(a.reshape(KC, 128, TOK).transpose(1, 0, 2))

        ins.append(dict(
            xh16=arrange_x(xTh),
            xl16=arrange_x(xTl),
            x32=arrange_x(xT),
            wkh=np.ascontiguousarray(wkTh),
            wkl=np.ascontiguousarray(wkTl),
            wq0=np.ascontiguousarray(wqT[:, :, :PW]),
            wq1=np.ascontiguousarray(wqT[:, :, PW:]),
            wv0=np.ascontiguousarray(wvT),
            wo16d=wo16d,
            cosF=cosF,
            sinF=sinF,
        ))
    return ins


def kernel(**inputs):
    x = np.asarray(inputs["x"], dtype=np.float32)
    wqkv_q = np.asarray(inputs["wqkv_q"])
    wqkv_scale = np.asarray(inputs["wqkv_scale"], dtype=np.float32)
    wo_q = np.asarray(inputs["wo_q"])
    wo_scale = np.asarray(inputs["wo_scale"], dtype=np.float32)
    start_pos = np.asarray(inputs["start_pos"])

    if "nc" not in _COMPILED:
        _COMPILED["nc"] = _build()
    nc = _COMPILED["nc"]

    in_maps = _prep_core_inputs(x, wqkv_q, wqkv_scale, wo_q, wo_scale, start_pos)
    res = run_bass_kernel_spmd(nc, in_maps, list(range(8)), trace=TRACE)
    global LAST_RESULTS
    LAST_RESULTS = res
    outs = [res.results[c]["outT"] for c in range(8)]
    full = np.empty((B * S, H), dtype=np.float32)
    for s in range(B):
        part = outs[2 * s] + outs[2 * s + 1]     # [H, TOK]
        full[s * S:(s + 1) * S, :] = part.T
    return full


if __name__ == "__main__":
    import reference as R
    import jax
    with jax.default_device(jax.devices("cpu")[0]):
        jin = R.setup_inputs()
        ref = np.asarray(R.reference(**jin))
        inp = {k: np.asarray(v) for k, v in jin.items()}
    out = kernel(**inp)
    rel = np.linalg.norm(out - ref) / np.linalg.norm(ref)
    print("Relative error:", rel)


# revision 31
# speedup vs baseline: 1.1218x; 1.1218x over previous
"""TRN2 Bass kernel for nn_Attention_43396349559334.

Prefill attention layer: B=4 seqs x S=1024, H=2048, 16 q heads / 8 kv heads
(GQA rep 2), HD=128, weight-only-quantized projections (group 128), KV int8
quant-dequant roundtrip (group 8 along head dim), interleaved RoPE, causal.

Sharding over 8 cores: core c = 2*s + t -> sequence s (data parallel over the
4 sequences), TP half t (8 q heads + 4 kv heads per core; row-parallel wo with
host-side partial sum over TP pairs).

Numerics: q and v projections and q@k' scores run as single fp32r matmuls
(~17 effective mantissa bits at full fp16 PE rate for >=256-col streams).
The k projection keeps fp16 hi/lo pair matmuls (21-bit): k feeds the int8
cache quantization whose rounding decisions amplify small errors by a full
quantization step, so ~1e-5 relative accuracy is required there. P@v' and
wo are single fp16. All weight dequantization and the x hi/lo split happen
on the host; the xl half streams from DRAM per chunk and is never resident.
"""
import math
import numpy as np
from contextlib import ExitStack

import concourse.bass as bass
import concourse.bacc as bacc
import concourse.mybir as mybir
import concourse.tile as tile
from concourse.bass_utils import run_bass_kernel_spmd
from concourse.masks import make_identity, make_causal_mask

dt = mybir.dt
F32, F16, I32, F32R = dt.float32, dt.float16, dt.int32, dt.float32r
AF = mybir.ActivationFunctionType
OP = mybir.AluOpType

B, S, H = 4, 1024, 2048
NH, NKV, HD = 16, 8, 128
WG, CG = 128, 8
ROPE_THETA = 10000.0
TOK = S                  # tokens per core (one sequence)
NHC, NKVC = NH // 2, NKV // 2   # per-core heads: 8 q, 4 kv
KC = H // 128            # 16 contraction chunks
TC = TOK // 128          # 8 token chunks
PW = 512                 # projection piece width (outs per piece)
INVSQ = 1.0 / math.sqrt(HD)
NEG = -1e30


def build_kernel(nc):
    """Emit the per-core kernel."""
    # k path: x hi/lo and w_k hi/lo in f16 (host pre-split / pre-dequantized)
    xh16 = nc.declare_dram_parameter("xh16", [128, KC, TOK], F16, isOutput=False)
    xl16 = nc.declare_dram_parameter("xl16", [128, KC, TOK], F16, isOutput=False)
    wkh = nc.declare_dram_parameter("wkh", [128, KC, NKVC * HD], F16, isOutput=False)
    wkl = nc.declare_dram_parameter("wkl", [128, KC, NKVC * HD], F16, isOutput=False)
    # q/v path: f32 operands for fp32r matmuls (host pre-dequantized)
    x32 = nc.declare_dram_parameter("x32", [128, KC, TOK], F32R, isOutput=False)
    wq0 = nc.declare_dram_parameter("wq0", [128, KC, PW], F32R, isOutput=False)
    wq1 = nc.declare_dram_parameter("wq1", [128, KC, PW], F32R, isOutput=False)
    wv0 = nc.declare_dram_parameter("wv0", [128, KC, PW], F32R, isOutput=False)
    # wo: host pre-dequantized f16
    wo16d = nc.declare_dram_parameter("wo16d", [128, H // 128, NHC, 128], F16, isOutput=False)
    cosF = nc.declare_dram_parameter("cosF", [TOK, HD // 2], F32, isOutput=False)
    sinF = nc.declare_dram_parameter("sinF", [TOK, HD // 2], F32, isOutput=False)
    outT = nc.declare_dram_parameter("outT", [H, TOK], F32, isOutput=True)

    with tile.TileContext(nc) as tc, ExitStack() as top:
        const_p = top.enter_context(tc.tile_pool(name="const", bufs=1))
        small_p = top.enter_context(tc.tile_pool(name="small", bufs=2))
        stage_p = top.enter_context(tc.tile_pool(name="stage", bufs=2))
        store_p = top.enter_context(tc.tile_pool(name="store", bufs=1))

        # ---------------- constants ----------------
        ident16 = const_p.tile([128, 128], F16)
        make_identity(nc, ident16[:])
        ident32 = const_p.tile([128, 128], F32)
        make_identity(nc, ident32[:])
        cmask = const_p.tile([128, 128], F32)
        make_causal_mask(nc, cmask[:], mask_val=NEG)
        cosT = const_p.tile([128, TC, HD // 2], F32)   # [tok128, tchunk, 64]
        sinT = const_p.tile([128, TC, HD // 2], F32)
        nc.sync.dma_start(cosT[:], cosF[:].rearrange("(t p) d -> p t d", p=128))
        nc.sync.dma_start(sinT[:], sinF[:].rearrange("(t p) d -> p t d", p=128))

        # long-lived stores
        kT32 = store_p.tile([128, NKVC, TOK], F32R, tag="kT", bufs=1)  # [HD, kvh, tok]
        v16 = store_p.tile([128, TC, NKVC * HD], F16, tag="v16", bufs=1)
        x32s = store_p.tile([128, KC, TOK], F32R, tag="x32s", bufs=1)
        qT32 = store_p.tile([128, NHC, TOK], F32R, tag="qT", bufs=1)   # [HD, head, tok]

        # ---------- helpers ----------
        def rope(acc, t, width, out_tag):
            """acc: [128, width] f32 (PSUM ok) -> rot [128, width] f32 SBUF."""
            nh = width // HD
            rot = stage_p.tile([128, PW], F32, tag=out_tag, name="rot")
            v4 = lambda ap: ap.rearrange("p (h d two) -> p h d two", h=nh, two=2)
            te, to = v4(acc[:, :width])[:, :, :, 0], v4(acc[:, :width])[:, :, :, 1]
            re, ro = v4(rot[:, :width])[:, :, :, 0], v4(rot[:, :width])[:, :, :, 1]
            cos = cosT[:, t, :].unsqueeze(1).broadcast_to([128, nh, HD // 2])
            sin = sinT[:, t, :].unsqueeze(1).broadcast_to([128, nh, HD // 2])
            t1 = stage_p.tile([128, PW // 2], F32, tag="rope_t1", bufs=1, name="t1")
            t2 = stage_p.tile([128, PW // 2], F32, tag="rope_t2", bufs=1, name="t2")
            t1v = t1[:, :width // 2].rearrange("p (h d) -> p h d", h=nh)
            t2v = t2[:, :width // 2].rearrange("p (h d) -> p h d", h=nh)
            nc.vector.tensor_tensor(out=t1v, in0=to, in1=sin, op=OP.mult)
            nc.vector.tensor_tensor(out=t2v, in0=te, in1=cos, op=OP.mult)
            nc.vector.tensor_tensor(out=re, in0=t2v, in1=t1v, op=OP.subtract)
            nc.vector.tensor_tensor(out=t1v, in0=te, in1=sin, op=OP.mult)
            nc.vector.tensor_tensor(out=t2v, in0=to, in1=cos, op=OP.mult)
            nc.vector.tensor_tensor(out=ro, in0=t1v, in1=t2v, op=OP.add)
            return rot

        def quant(x32t, width, out_ap):
            """x32t: f32 tile [128, >=width]; out_ap: [128, ng, CG] view."""
            ng = width // CG
            xg = x32t[:, :width].rearrange("p (g c) -> p g c", c=CG)
            amax = small_p.tile([128, PW // CG], F32, tag="amax", name="amax")
            nc.vector.tensor_reduce(amax[:, :ng], xg, axis=mybir.AxisListType.X,
                                    op=OP.max, apply_absolute_value=True)
            s = small_p.tile([128, PW // CG], F32, tag="qs", name="s")
            nc.vector.tensor_scalar(out=s[:, :ng], in0=amax[:, :ng], scalar1=1.0 / 127.0,
                                    scalar2=1e-8, op0=OP.mult, op1=OP.add)
            rinv = small_p.tile([128, PW // CG], F32, tag="qrinv", name="rinv")
            nc.vector.reciprocal(rinv[:, :ng], s[:, :ng])
            y = stage_p.tile([128, PW], F32, tag="qy", name="y")
            nc.vector.tensor_tensor(out=y[:, :width].rearrange("p (g c) -> p g c", c=CG),
                                    in0=xg,
                                    in1=rinv[:, :ng].unsqueeze(2).broadcast_to([128, ng, CG]),
                                    op=OP.mult)
            lev = stage_p.tile([128, PW], I32, tag="qlev", bufs=1, name="lev")
            nc.scalar.copy(lev[:, :width], y[:, :width])
            levf = stage_p.tile([128, PW], F32, tag="qy", name="levf")
            nc.scalar.copy(levf[:, :width], lev[:, :width])
            nc.vector.tensor_tensor(out=out_ap,
                                    in0=levf[:, :width].rearrange("p (g c) -> p g c", c=CG),
                                    in1=s[:, :ng].unsqueeze(2).broadcast_to([128, ng, CG]),
                                    op=OP.mult)

        # ============ phases K and QV share psum pools ============
        with tc.tile_pool(name="ps_acc", bufs=1, space="PSUM") as ps_acc, \
             tc.tile_pool(name="ps_tr", bufs=1, space="PSUM") as ps_tr:
            # warm the PE (p-state ramp) while the first input DMAs land
            # (borrows the QV accumulator slots, long free by the time QV runs)
            junk = ps_acc.tile([128, PW], F32, tag="accq", bufs=2, name="junk")
            for _ in range(10):
                nc.tensor.matmul(junk[:, :128], ident32[:], ident32[:],
                                 start=True, stop=True)

            # ---- phase K: k projection, f16 hi/lo 3mm, g-outer over t-halves.
            # x hi/lo stream from DRAM in 4-chunk quarters in exact consumption
            # order (few, large DMA triggers: the sync engine issues triggers
            # serially, so trigger count and order gate the pipeline start).
            with tc.tile_pool(name="kphase", bufs=1) as kp:
                wkh_s = kp.tile([128, KC, NKVC * HD], F16)
                wkl_s = kp.tile([128, KC, NKVC * HD], F16)

                for half in range(2):
                    h0 = half * 512
                    ts = list(range(half * 4, half * 4 + 4))
                    accs = {}
                    for t in ts:
                        accs[t] = ps_acc.tile([128, PW], F32, tag="acck", bufs=4,
                                              name=f"acc{t}")
                    for gq in range(4):
                        gs = slice(gq * 4, gq * 4 + 4)
                        xhq = kp.tile([128, 4, 512], F16, tag="xhq", bufs=2, name="xhq")
                        xlq = kp.tile([128, 4, 512], F16, tag="xlq", bufs=2, name="xlq")
                        if half == 0:
                            nc.sync.dma_start(wkh_s[:, gs, :], wkh[:, gs, :])
                        nc.sync.dma_start(xhq[:], xh16[:, gs, h0:h0 + 512])
                        nc.sync.dma_start(xlq[:], xl16[:, gs, h0:h0 + 512])
                        if half == 0:
                            nc.sync.dma_start(wkl_s[:, gs, :], wkl[:, gs, :])
                        if half == 0:
                            # prefetch the fp32r x for phase QV behind phase-K input
                            nc.sync.dma_start(x32s[:, gs, :], x32[:, gs, :])
                        n = KC * 3
                        for gi in range(4):
                            g = gq * 4 + gi
                            for ti, t in enumerate(ts):
                                i = g * 3
                                lx_h = xhq[:, gi, ti * 128:(ti + 1) * 128]
                                lx_l = xlq[:, gi, ti * 128:(ti + 1) * 128]
                                nc.tensor.matmul(accs[t][:], lx_h, wkh_s[:, g, :],
                                                 start=(i == 0), stop=(i == n - 1))
                                nc.tensor.matmul(accs[t][:], lx_h, wkl_s[:, g, :],
                                                 start=False, stop=(i + 1 == n - 1))
                                nc.tensor.matmul(accs[t][:], lx_l, wkh_s[:, g, :],
                                                 start=False, stop=(i + 2 == n - 1))
                    for t in ts:
                        rot = rope(accs[t], t, PW, "rot")
                        ktmp = stage_p.tile([128, PW], F32, tag="ktmp", bufs=1, name="ktmp")
                        quant(rot, PW, ktmp[:].rearrange("p (g c) -> p g c", c=CG))
                        pt = ps_tr.tile([128, PW], F32, tag="pt", bufs=2, name="pt")
                        for j in range(NKVC):
                            nc.tensor.transpose(pt[:, j * 128:(j + 1) * 128],
                                                ktmp[:, j * 128:(j + 1) * 128], ident32[:])
                        nc.vector.tensor_copy(
                            kT32[:, :, t * 128:(t + 1) * 128],
                            pt[:].rearrange("p (j f) -> p j f", j=NKVC))

            # ---- phase QV: q/v projections (fp32r) ----
            with tc.tile_pool(name="wpiece", bufs=1) as w_p:
                pieces = [("q", 0, wq0), ("q", 1, wq1), ("v", 0, wv0)]
                for kind, p, w_dram in pieces:
                    wp = w_p.tile([128, KC, PW], F32R, tag="w32", bufs=2, name="wp")
                    nc.sync.dma_start(wp[:, :KC // 2, :], w_dram[:, :KC // 2, :])
                    nc.sync.dma_start(wp[:, KC // 2:, :], w_dram[:, KC // 2:, :])
                    for t in range(TC):
                        acc = ps_acc.tile([128, PW], F32, tag="accq", bufs=2, name="acc")
                        for g in range(KC):
                            nc.tensor.matmul(
                                acc[:],
                                x32s[:, g, t * 128:(t + 1) * 128],
                                wp[:, g, :],
                                start=(g == 0), stop=(g == KC - 1))
                        if kind == "q":
                            rot = rope(acc, t, PW, "rot")
                            pt = ps_tr.tile([128, PW], F32, tag="pt", bufs=2, name="pt")
                            for j in range(PW // 128):
                                nc.tensor.transpose(pt[:, j * 128:(j + 1) * 128],
                                                    rot[:, j * 128:(j + 1) * 128],
                                                    ident32[:])
                            nc.vector.tensor_copy(
                                qT32[:, 4 * p:4 * p + 4, t * 128:(t + 1) * 128],
                                pt[:].rearrange("p (j f) -> p j f", j=4))
                        else:
                            vstage = stage_p.tile([128, PW], F32, tag="rot", name="vstage")
                            nc.scalar.copy(vstage[:], acc[:])
                            quant(vstage, PW,
                                  v16[:, t, :].rearrange("p (g c) -> p g c", c=CG))

        # ============ phase A: attention with interleaved WO ============
        with tc.tile_pool(name="attnT", bufs=1) as at_p, \
             tc.tile_pool(name="wow", bufs=2) as wo_p:
            attnT = at_p.tile([128, NHC, TOK], F16)
            with tc.tile_pool(name="probs", bufs=3) as p_p, \
                 tc.tile_pool(name="ps_sc", bufs=2, space="PSUM") as psum_s, \
                 tc.tile_pool(name="ps_av", bufs=1, space="PSUM") as psum_v, \
                 tc.tile_pool(name="ps_pt", bufs=2, space="PSUM") as psum_pt, \
                 tc.tile_pool(name="ps_wo", bufs=1, space="PSUM") as psum_w:

                def attention_head(h, qi, pts_pair, h01, rsum2):
                    hkv = h // 2
                    L = (qi + 1) * 128
                    sc = psum_s.tile([128, TOK], F32, tag="scores", name="sc")
                    lq = qT32[:, h, qi * 128:(qi + 1) * 128]
                    for ci in range((L + 511) // 512):
                        c0, c1 = ci * 512, min(L, ci * 512 + 512)
                        nc.tensor.matmul(sc[:, c0:c1], lq, kT32[:, hkv, c0:c1],
                                         start=True, stop=True)
                    nc.vector.tensor_tensor(out=sc[:, L - 128:L], in0=sc[:, L - 128:L],
                                            in1=cmask[:], op=OP.add)
                    negm = small_p.tile([128, 1], F32, tag="negm", name="negm")
                    nc.vector.tensor_reduce(negm[:], sc[:, :L], axis=mybir.AxisListType.X,
                                            op=OP.max, negate=True)
                    bias = small_p.tile([128, 1], F32, tag="bias", name="bias")
                    nc.vector.tensor_scalar(out=bias[:], in0=negm[:], scalar1=INVSQ,
                                            scalar2=None, op0=OP.mult)
                    p16u = p_p.tile([128, TOK], F16, tag="p16u", bufs=4, name="p16u")
                    nc.scalar.activation(p16u[:, :L], sc[:, :L], AF.Exp,
                                         bias=bias[:], scale=INVSQ,
                                         accum_out=rsum2[:, h01:h01 + 1])
                    return p16u

                def normalize_transpose(p16u, qi, pts_pair, h01, rinv2):
                    L = (qi + 1) * 128
                    p16 = p_p.tile([128, TOK], F16, tag="p16", name="p16")
                    nc.vector.tensor_scalar(out=p16[:, :L], in0=p16u[:, :L],
                                            scalar1=rinv2[:, h01:h01 + 1],
                                            scalar2=None, op0=OP.mult)
                    ptp = psum_pt.tile([128, TC * 128], F16, tag="ptp", name="ptp")
                    for kc in range(qi + 1):
                        nc.tensor.transpose(ptp[:, kc * 128:(kc + 1) * 128],
                                            p16[:, kc * 128:(kc + 1) * 128], ident16[:])
                    nc.scalar.copy(
                        pts_pair[:, :qi + 1, h01, :],
                        ptp[:, :L].rearrange("p (k f) -> p k f", f=128))

                def wo_seg(hc, c0, w):
                    wo16 = wo_p.tile([128, NHC, 128], F16, tag="wo16", name="wo16")
                    nc.sync.dma_start(wo16[:], wo16d[:, hc, :, :])
                    po = psum_w.tile([128, 512], F32, tag="wo_out", name="po")
                    for g in range(NHC):
                        nc.tensor.matmul(po[:, :w], wo16[:, g, :],
                                         attnT[:, g, c0:c0 + w],
                                         start=(g == 0), stop=(g == NHC - 1))
                    pos = wo_p.tile([128, 512], F32, tag="wo_stage", bufs=3, name="pos")
                    nc.scalar.copy(pos[:, :w], po[:, :w])
                    nc.sync.dma_start(
                        outT[hc * 128:(hc + 1) * 128, c0:c0 + w], pos[:, :w])

                unit = 0
                for qi in (4, 5, 6, 7, 0, 1, 2, 3):
                    for hp in range(NHC // 2):
                        pts_pair = p_p.tile([128, TC, 2, 128], F16, tag="ptsp",
                                            bufs=3, name="pts_pair")
                        rsum2 = small_p.tile([128, 2], F32, tag="rsum", name="rsum2")
                        pa = attention_head(2 * hp, qi, pts_pair, 0, rsum2)
                        pb = attention_head(2 * hp + 1, qi, pts_pair, 1, rsum2)
                        rinv2 = small_p.tile([128, 2], F32, tag="rinv", name="rinv2")
                        nc.vector.reciprocal(rinv2[:], rsum2[:])
                        normalize_transpose(pa, qi, pts_pair, 0, rinv2)
                        normalize_transpose(pb, qi, pts_pair, 1, rinv2)
                        avp = psum_v.tile([128, 2, 128], F32, tag="avp", name="avp")
                        for kc in range(qi + 1):
                            nc.tensor.matmul(
                                avp[:].rearrange("p a b -> p (a b)"),
                                v16[:, kc, hp * HD:(hp + 1) * HD],
                                pts_pair[:, kc, :, :].rearrange("p a b -> p (a b)"),
                                start=(kc == 0), stop=(kc == qi))
                        nc.vector.tensor_copy(
                            attnT[:, 2 * hp:2 * hp + 2, qi * 128:(qi + 1) * 128],
                            avp[:])
                        # qi 4-7 ran first, so cols 512-1023 are final once the
                        # thin (qi 0-3) units start: spread WO half-1 through them
                        if qi < 4:
                            wo_seg(unit, 512, 512)
                            unit += 1
            # WO token-half-0 tail with the attention psum banks freed
            with tc.tile_pool(name="ps_wo2", bufs=3, space="PSUM") as psum_w:
                for hc in range(H // 128):
                    wo_seg(hc, 0, 512)


# ====================== host side ======================

_COMPILED = {}
TRACE = False
LAST_RESULTS = None


def _build():
    nc = bacc.Bacc("TRN2", target_bir_lowering=False, debug=False, num_devices=8)
    build_kernel(nc)
    nc.compile()
    return nc


def _prep_core_inputs(x, wqkv_q, wqkv_scale, wo_q, wo_scale, start_pos):
    """Build the 8 per-core input maps (numpy marshaling only)."""
    ins = []
    inv_freq = 1.0 / (ROPE_THETA ** (np.arange(0, HD, 2, dtype=np.float64) / HD))

    # full dequantized wqkv / wo in f32
    wqkv32 = wqkv_q.astype(np.float32) * np.repeat(wqkv_scale.astype(np.float32), WG, axis=1)
    wo32 = wo_q.astype(np.float32) * np.repeat(wo_scale.astype(np.float32), WG, axis=1)

    def arrange_T(a):
        """[out, H] f32 -> w^T arranged [128, KC, out] (partition = in-dim within chunk)."""
        aT = np.ascontiguousarray(a.T)                  # [H, out]
        return np.ascontiguousarray(
            aT.reshape(KC, 128, a.shape[0]).transpose(1, 0, 2))

    for c in range(8):
        s, t = c // 2, c % 2
        pos = (float(start_pos[s]) + np.arange(S, dtype=np.float64))[:, None] * inv_freq[None, :]
        cosF = np.cos(pos).astype(np.float32)
        sinF = np.sin(pos).astype(np.float32)

        xs = x[s * S:(s + 1) * S, :]                    # [1024, 2048]
        xT = np.ascontiguousarray(xs.T.astype(np.float32))   # [2048, 1024]

        qrows = slice(t * NHC * HD, (t + 1) * NHC * HD)
        krows = slice(NH * HD + t * NKVC * HD, NH * HD + (t + 1) * NKVC * HD)
        vrows = slice((NH + NKV) * HD + t * NKVC * HD, (NH + NKV) * HD + (t + 1) * NKVC * HD)

        wqT = arrange_T(wqkv32[qrows])                  # [128, KC, 1024]
        wkT = arrange_T(wqkv32[krows])                  # [128, KC, 512]
        wvT = arrange_T(wqkv32[vrows])                  # [128, KC, 512]
        wkTh = wkT.astype(np.float16)
        wkTl = (wkT - wkTh.astype(np.float32)).astype(np.float16)

        wo_cols = wo32[:, t * NHC * HD:(t + 1) * NHC * HD]      # [H, 1024]
        wo_T = wo_cols.T.astype(np.float16)                     # [1024, H]
        wo16d = np.ascontiguousarray(
            wo_T.reshape(NHC, 128, H // 128, 128).transpose(1, 2, 0, 3))  # [128, 16, 8, 128]

        xTh = xT.astype(np.float16)
        xTl = (xT - xTh.astype(np.float32)).astype(np.float16)

        def arrange_x(a):                               # [H, TOK] -> [128, KC, TOK]
            return np.ascontiguousarray(a.reshape(KC, 128, TOK).transpose(1, 0, 2))

        ins.append(dict(
            xh16=arrange_x(xTh),
            xl16=arrange_x(xTl),
            x32=arrange_x(xT),
            wkh=np.ascontiguousarray(wkTh),
            wkl=np.ascontiguousarray(wkTl),
            wq0=np.ascontiguousarray(wqT[:, :, :PW]),
            wq1=np.ascontiguousarray(wqT[:, :, PW:]),
            wv0=np.ascontiguousarray(wvT),
            wo16d=wo16d,
            cosF=cosF,
            sinF=sinF,
        ))
    return ins


def kernel(**inputs):
    x = np.asarray(inputs["x"], dtype=np.float32)
    wqkv_q = np.asarray(inputs["wqkv_q"])
    wqkv_scale = np.asarray(inputs["wqkv_scale"], dtype=np.float32)
    wo_q = np.asarray(inputs["wo_q"])
    wo_scale = np.asarray(inputs["wo_scale"], dtype=np.float32)
    start_pos = np.asarray(inputs["start_pos"])

    if "nc" not in _COMPILED:
        _COMPILED["nc"] = _build()
    nc = _COMPILED["nc"]

    in_maps = _prep_core_inputs(x, wqkv_q, wqkv_scale, wo_q, wo_scale, start_pos)
    res = run_bass_kernel_spmd(nc, in_maps, list(range(8)), trace=TRACE)
    global LAST_RESULTS
    LAST_RESULTS = res
    outs = [res.results[c]["outT"] for c in range(8)]
    full = np.empty((B * S, H), dtype=np.float32)
    for s in range(B):
        part = outs[2 * s] + outs[2 * s + 1]     # [H, TOK]
        full[s * S:(s + 1) * S, :] = part.T
    return full


if __name__ == "__main__":
    import reference as R
    import jax
    with jax.default_device(jax.devices("cpu")[0]):
        jin = R.setup_inputs()
        ref = np.asarray(R.reference(**jin))
        inp = {k: np.asarray(v) for k, v in jin.items()}
    out = kernel(**inp)
    rel = np.linalg.norm(out - ref) / np.linalg.norm(ref)
    print("Relative error:", rel)
